# revision 71
# baseline (speedup 1.0000x reference)
"""Sliding-window (banded) attention for nn_AttLayer on 8 Trainium2 NeuronCores.

Reference computation (per window-block n of 512 positions, 64 blocks over L=32768):
  q/k/v = 1x1-conv projections of x1 (512ch -> 256ch)
  energy[l, m] = (q_block[:, l] . k_window[:, m]) / 16   over a 1024-wide window
  attn = softmax(energy + log(band_mask + 1e-6)) * band_mask
  out  = relu(v_window @ attn^T) -> 1x1-conv (256 -> 512) + bias, masked

Sharding: 64 blocks split contiguously across 8 cores (8 blocks each). Each core
gets a zero-padded halo slice of x1 and computes its 4096 output columns.

Kernel strategy (per core, SPMD — all per-core variation is in the data):
  - Projections computed on PE with float32r (fp32 with 12-bit significand;
    matmul is exact for pre-rounded inputs). q/k natural layout (c on
    partitions), v projected directly TRANSPOSED (positions on partitions) so
    the attention AV matmul needs no transposes.
  - energy computed transposed: energyT[m, l] = k_chunk^T q  (PE), only over
    the ~62% of 128x128 tiles that intersect the band (padded to N>=256).
  - Band masking (MASK_MODE="gpsimd"): affine_select on the otherwise-idle
    GPSIMD engine zeroes out-of-band exp values; sequence-edge padding is
    handled with per-core 0/1 data vectors so the program stays SPMD.
    (MASK_MODE="pe" alternative: additive ln(1e-6) mask preloaded into PSUM
    via a bf16 identity matmul, exactly matching the reference's +1e-6 terms.)
  - exp on ScalarE (free scale=1/16), denominators via an all-ones f32r matmul
    (column sums land replicated across partitions), reciprocal on VectorE.
  - AV + output projection on PE; normalization fused into the PSUM->SBUF
    eviction with scalar_tensor_tensor. Final bias/mask applied on host.
  - Blocks are software-pipelined (block b's colsum/AV/outproj emitted after
    block b+1's energy+exp) so PE never waits on the softmax chain, and the
    k/q/vT projection groups are interleaved INTO the block stream: the
    k-projection alone would consume x1 above the HBM wire rate, so each
    fresh-x-hungry k group is followed by work on already-resident data.
  - Halo reuse between the two halves: half 1's first four vT tiles alias
    half 0's last four (identical x1 columns), and half 0's k overlap is
    stashed via an SBUF->SBUF DMA so half 1 skips its first k-group.
"""

import numpy as np

NCORES = 8
L = 32768
CIN = 512
C = 256
BL = 512
HALF = 256
LC = L // NCORES              # 4096 positions per core
HALO = LC + 2 * HALF          # 4608
NBH = 2                       # halves per core
LH = LC // NBH                # 2048 positions per half
KSPAN = LH + 2 * HALF         # 2560 k/v positions per half
BPH = 4                       # blocks per half
SCALE = 1.0 / 16.0
NEG = float(np.log(1e-6) / SCALE)   # additive raw-energy mask ~= -221.048

# Per m-chunk r' (8 chunks of the 1024-wide window): padded valid l-interval
# (lo, width) within the block's 512 queries, all widths >= 256 for f32r speed.
INTERVALS = [
    (0, 256), (0, 256), (0, 384), (0, 512),
    (0, 512), (128, 384), (256, 256), (256, 256),
]
MOFF = np.cumsum([0] + [w for _, w in INTERVALS]).tolist()  # offsets into mT_int
MTOT = MOFF[-1]  # 2816
# accumulation order: r'=3 covers the full [0,512) so it goes first (start=True)
AVORDER = [3, 4, 2, 5, 1, 6, 0, 7]
# "pe": additive log-mask preloaded into PSUM via bf16 identity matmul.
# "gpsimd": band-mask applied post-exp with affine_select on the idle Pool
#           engine (masked terms become exact zeros); edges via per-core data.
MASK_MODE = "gpsimd"


def _round_f32r(x):
    # round-to-nearest into the f32r grid (fp32 with low 12 mantissa bits zero)
    b = np.ascontiguousarray(x, dtype=np.float32).view(np.uint32)
    return ((b + np.uint32(0x800)) & np.uint32(0xFFFFF000)).view(np.float32)


def _mask_tile(r, lo, w, all_pad):
    if all_pad:
        return np.full((128, w), NEG, dtype=np.float32)
    m = np.arange(128 * r, 128 * r + 128, dtype=np.int64)[:, None]
    l = np.arange(lo, lo + w, dtype=np.int64)[None, :]
    valid = (m - l >= 0) & (m - l <= BL - 1)
    return np.where(valid, 0.0, NEG).astype(np.float32)


def _build_program():
    import concourse.mybir as mybir
    from concourse import bacc
    from concourse.tile import TileContext

    F32 = mybir.dt.float32
    F32R = mybir.dt.float32r
    BF16 = mybir.dt.bfloat16
    Alu = mybir.AluOpType
    Act = mybir.ActivationFunctionType

    nc = bacc.Bacc()

    x1h_d = nc.dram_tensor("x1h", [CIN, HALO], F32R, kind="ExternalInput")
    wqT_d = nc.dram_tensor("wqT", [CIN, C], F32R, kind="ExternalInput")
    wkT_d = nc.dram_tensor("wkT", [CIN, C], F32R, kind="ExternalInput")
    wvT_d = nc.dram_tensor("wvT", [CIN, C], F32R, kind="ExternalInput")
    woT_d = nc.dram_tensor("woT", [C, CIN], F32R, kind="ExternalInput")
    bq_d = nc.dram_tensor("bq2", [2, 128, 1], F32, kind="ExternalInput")
    bk_d = nc.dram_tensor("bk2", [2, 128, 1], F32, kind="ExternalInput")
    bvr_d = nc.dram_tensor("bvr", [128, C], F32, kind="ExternalInput")
    ones_d = nc.dram_tensor("ones", [128, 128], F32R, kind="ExternalInput")
    if MASK_MODE == "pe":
        ident_d = nc.dram_tensor("ident", [128, 128], BF16, kind="ExternalInput")
        mint_d = nc.dram_tensor("mT_int", [128, MTOT], BF16, kind="ExternalInput")
        mfirst_d = nc.dram_tensor("mT_first", [128, 512], BF16, kind="ExternalInput")
        mlast_d = nc.dram_tensor("mT_last", [128, 512], BF16, kind="ExternalInput")
    else:
        padf_d = nc.dram_tensor("padf", [2, 128, 1], F32, kind="ExternalInput")
        padl_d = nc.dram_tensor("padl", [2, 128, 1], F32, kind="ExternalInput")
    out_d = nc.dram_tensor("out", [CIN, LC], F32, kind="ExternalOutput")

    with TileContext(nc) as tc:
        with (
            tc.tile_pool(name="consts", bufs=1) as consts,
            tc.tile_pool(name="xpool", bufs=1) as xpool,
            tc.tile_pool(name="qkv", bufs=1) as qkv,
            tc.tile_pool(name="ptp", bufs=2) as ptp,
            tc.tile_pool(name="sbo", bufs=4) as sbo,
            tc.tile_pool(name="pse", bufs=3, space="PSUM") as pse,
            tc.tile_pool(name="pss", bufs=1, space="PSUM") as pss,
            tc.tile_pool(name="psav", bufs=1, space="PSUM") as psav,
            tc.tile_pool(name="pso", bufs=2, space="PSUM") as pso,
        ):
            # warm the ACT exp table while DMAs stream in
            warm_sb = consts.tile([1, 8], F32)
            nc.vector.memset(warm_sb, 0.0)
            nc.scalar.activation(warm_sb, warm_sb, Act.Exp)

            # warm the PE clock gate (HAM) during the initial DMA wait:
            # dummy bf16 matmuls on memset data keep the array busy so the
            # first real projections run at the full 2.4 GHz
            warm_a = consts.tile([128, 128], BF16, name="warm_a")
            nc.vector.memset(warm_a, 1.0)
            warm_b = consts.tile([128, 512], BF16, name="warm_b")
            nc.vector.memset(warm_b, 1.0)
            for wi in range(5):
                warm_ps = pse.tile([128, 512], F32, tag="e", name=f"wps{wi}")
                nc.tensor.matmul(warm_ps, warm_a, warm_b, start=True, stop=True)

            # critical-path-first DMA order: the first PE work is the h=0
            # k-projection of columns [0:512), needing wkT/bk and x chunk 0;
            # pair (wkT[kc], x[kc]) so the accumulation group streams in
            wT_sb = {}
            x_sb_h0 = []
            for kc in range(4):
                t = consts.tile([128, C], F32R, name=f"wkT{kc}")
                nc.sync.dma_start(out=t, in_=wkT_d.ap()[128 * kc:128 * (kc + 1), :])
                wT_sb[("k", kc)] = t
                tx = xpool.tile([128, KSPAN], F32R, tag=f"x{kc}", name=f"x{kc}_0")
                x_sb_h0.append(tx)
                nc.sync.dma_start(
                    out=tx[:, 0:512],
                    in_=x1h_d.ap()[128 * kc:128 * (kc + 1), 0:512],
                )
            bk_sb = []
            for cc in range(2):
                tk = consts.tile([128, 1], F32, name=f"bk{cc}")
                nc.sync.dma_start(out=tk, in_=bk_d.ap()[cc])
                bk_sb.append(tk)
            def _x0_chunk(ct):
                for kc in range(4):
                    nc.sync.dma_start(
                        out=x_sb_h0[kc][:, 512 * ct:512 * (ct + 1)],
                        in_=x1h_d.ap()[128 * kc:128 * (kc + 1),
                                       512 * ct:512 * (ct + 1)],
                    )

            _x0_chunk(1)
            for kc in range(4):
                t = consts.tile([128, C], F32R, name=f"wqT{kc}")
                nc.sync.dma_start(out=t, in_=wqT_d.ap()[128 * kc:128 * (kc + 1), :])
                wT_sb[("q", kc)] = t
            bq_sb = []
            for cc in range(2):
                tq = consts.tile([128, 1], F32, name=f"bq{cc}")
                nc.sync.dma_start(out=tq, in_=bq_d.ap()[cc])
                bq_sb.append(tq)
            for kc in range(4):
                t = consts.tile([128, C], F32R, name=f"wvT{kc}")
                nc.sync.dma_start(out=t, in_=wvT_d.ap()[128 * kc:128 * (kc + 1), :])
                wT_sb[("v", kc)] = t
            bvrep_sb = consts.tile([128, C], F32)
            nc.sync.dma_start(out=bvrep_sb, in_=bvr_d.ap())
            _x0_chunk(2)
            _x0_chunk(3)
            _x0_chunk(4)

            ones_sb = consts.tile([128, 128], F32R)
            nc.sync.dma_start(out=ones_sb, in_=ones_d.ap())
            if MASK_MODE == "pe":
                ident_sb = consts.tile([128, 128], BF16)
                nc.sync.dma_start(out=ident_sb, in_=ident_d.ap())
                mint_sb = consts.tile([128, MTOT], BF16)
                nc.sync.dma_start(out=mint_sb, in_=mint_d.ap())
                mfirst_sb = consts.tile([128, 512], BF16)
                nc.sync.dma_start(out=mfirst_sb, in_=mfirst_d.ap())
                mlast_sb = consts.tile([128, 512], BF16)
                nc.sync.dma_start(out=mlast_sb, in_=mlast_d.ap())
            else:
                padf_sb, padl_sb = [], []
                for r in range(2):
                    tf = consts.tile([128, 1], F32, name=f"padf{r}")
                    nc.sync.dma_start(out=tf, in_=padf_d.ap()[r])
                    padf_sb.append(tf)
                    tl = consts.tile([128, 1], F32, name=f"padl{r}")
                    nc.sync.dma_start(out=tl, in_=padl_d.ap()[r])
                    padl_sb.append(tl)
            woT_sb = []
            for cc in range(2):
                t = consts.tile([128, CIN], F32R, name=f"woT{cc}")
                nc.sync.dma_start(out=t, in_=woT_d.ap()[128 * cc:128 * (cc + 1), :])
                woT_sb.append(t)

            for h in range(NBH):
                base = LH * h  # halo-coord start of this half's x1/k/v span
                if h == 0:
                    x_sb = x_sb_h0
                else:
                    x_sb = []
                    for kc in range(4):
                        t = xpool.tile([128, KSPAN], F32R, tag=f"x{kc}",
                                       name=f"x{kc}_{h}")
                        x_sb.append(t)
                # split per 512-column chunk so projections start while the
                # rest of the slice streams in (all h=0 chunks issued up top)
                if h > 0:
                    for kc in range(4):
                        nc.sync.dma_start(
                            out=x_sb[kc][:, 256:512],
                            in_=x1h_d.ap()[128 * kc:128 * (kc + 1),
                                           base + 256:base + 512],
                        )
                    for ct in range(1, KSPAN // 512):
                        for kc in range(4):
                            nc.sync.dma_start(
                                out=x_sb[kc][:, 512 * ct:512 * (ct + 1)],
                                in_=x1h_d.ap()[128 * kc:128 * (kc + 1),
                                               base + 512 * ct:base + 512 * (ct + 1)],
                            )

                # ---- projections ----
                q_sb, k_sb = [], []
                for cc in range(2):
                    q_sb.append(qkv.tile([128, LH], F32R, tag=f"q{cc}", name=f"q{cc}_{h}"))
                    k_sb.append(qkv.tile([128, KSPAN], F32R, tag=f"k{cc}", name=f"k{cc}_{h}"))
                # projection group emitters; actual emission is interleaved
                # with the attention blocks below so the k-projection's burst
                # demand for fresh x chunks never outruns the DMA wire rate
                def k_group(mt):
                    for cc in range(2):
                        csl = slice(128 * cc, 128 * (cc + 1))
                        ps = pse.tile([128, 512], F32, tag="e",
                                      name=f"psk{h}{cc}{mt}")
                        for kc in range(4):
                            nc.tensor.matmul(
                                ps, wT_sb[("k", kc)][:, csl],
                                x_sb[kc][:, 512 * mt:512 * (mt + 1)],
                                start=(kc == 0), stop=(kc == 3),
                            )
                        nc.vector.tensor_scalar_add(
                            k_sb[cc][:, 512 * mt:512 * (mt + 1)], ps, bk_sb[cc]
                        )

                def q_group(lt):
                    for cc in range(2):
                        csl = slice(128 * cc, 128 * (cc + 1))
                        ps = pse.tile([128, 512], F32, tag="e",
                                      name=f"psq{h}{cc}{lt}")
                        for kc in range(4):
                            nc.tensor.matmul(
                                ps, wT_sb[("q", kc)][:, csl],
                                x_sb[kc][:, HALF + 512 * lt: HALF + 512 * (lt + 1)],
                                start=(kc == 0), stop=(kc == 3),
                            )
                        nc.vector.tensor_scalar_add(
                            q_sb[cc][:, 512 * lt:512 * (lt + 1)], ps, bq_sb[cc]
                        )

                vT_sb = [None] * (KSPAN // 128)
                if h > 0:
                    # halo reuse: this half's m=0..3 v-chunks cover the same
                    # x1 columns as the previous half's m=16..19 — alias them
                    for mt in range(4):
                        vT_sb[mt] = prev_vT[16 + mt]

                def vT_group(mts):
                    for mt in mts:
                        ps = pso.tile([128, C], F32, tag="o", name=f"psv{h}{mt}")
                        for kc in range(4):
                            nc.tensor.matmul(
                                ps, x_sb[kc][:, 128 * mt:128 * (mt + 1)],
                                wT_sb[("v", kc)], start=(kc == 0), stop=(kc == 3),
                            )
                        t = qkv.tile([128, C], F32R, tag=f"v{mt}", name=f"vT{mt}_{h}")
                        # eviction with the (per-free-element) v bias folded in
                        nc.vector.tensor_tensor(t, ps, bvrep_sb, op=Alu.add)
                        vT_sb[mt] = t

                # ---- attention blocks (software-pipelined: block b's
                # colsum/AV/outproj are emitted after block b+1's energy+exp
                # so PE never waits on the ACT/Pool softmax chain) ----
                def emit_energy(h, b, k_sb=k_sb, q_sb=q_sb, vT_sb=vT_sb,
                                kh=(kh_prev if h > 0 else None)):
                    woff = 512 * b   # window start in k/vT coords
                    first_blk = (h == 0 and b == 0)
                    last_blk = (h == NBH - 1 and b == BPH - 1)
                    pt = {}
                    for r in AVORDER:
                        lo, w = INTERVALS[r]
                        ps_e = pse.tile([128, w], F32, tag="e", name=f"pse{h}{b}{r}")
                        if MASK_MODE == "pe":
                            if first_blk and r < 2:
                                msrc = mfirst_sb[:, 256 * r:256 * r + w]
                            elif last_blk and r >= 6:
                                msrc = mlast_sb[:, 256 * (r - 6):256 * (r - 6) + w]
                            else:
                                msrc = mint_sb[:, MOFF[r]:MOFF[r] + w]
                            nc.tensor.matmul(ps_e, ident_sb, msrc, start=True,
                                             stop=False, skip_group_check=True)
                        for cc in range(2):
                            if kh is not None and b == 0 and r < 4:
                                klhs = kh[cc][:, 128 * r:128 * (r + 1)]
                            else:
                                klhs = k_sb[cc][:, woff + 128 * r:
                                                woff + 128 * (r + 1)]
                            nc.tensor.matmul(
                                ps_e, klhs,
                                q_sb[cc][:, 512 * b + lo: 512 * b + lo + w],
                                start=(MASK_MODE != "pe" and cc == 0),
                                stop=(cc == 1), skip_group_check=True,
                            )
                        t = ptp.tile([128, w], F32R, tag=f"pt{r}", name=f"pt{r}_{h}{b}")
                        nc.scalar.activation(t, ps_e, Act.Exp, scale=SCALE)
                        if MASK_MODE == "gpsimd":
                            # zero outside the band: one affine compare per tile
                            # (lower bound bites for r<=3, upper for r>=4)
                            if r <= 3:
                                nc.gpsimd.affine_select(
                                    out=t, in_=t, compare_op=Alu.is_ge, fill=0.0,
                                    base=128 * r - lo, channel_multiplier=1,
                                    pattern=[[-1, w]],
                                )
                            else:
                                # valid iff (128r+m')-l <= 511, recast as
                                # (511-128r+lo) - m' + j >= 0 (is_ge only)
                                nc.gpsimd.affine_select(
                                    out=t, in_=t, compare_op=Alu.is_ge, fill=0.0,
                                    base=(BL - 1) - 128 * r + lo,
                                    channel_multiplier=-1,
                                    pattern=[[1, w]],
                                )
                            if first_blk and r < 2:
                                nc.vector.tensor_scalar_mul(t, t, padf_sb[r])
                            elif last_blk and r >= 6:
                                nc.vector.tensor_scalar_mul(t, t, padl_sb[r - 6])
                        pt[r] = t
                    return (h, b, pt, vT_sb)

                def emit_tail(ctx):
                    h, b, pt, vT_l = ctx
                    ps_s = pss.tile([128, 512], F32, tag="s", name=f"pss{h}{b}")
                    for i, r in enumerate(AVORDER):
                        lo, w = INTERVALS[r]
                        nc.tensor.matmul(
                            ps_s[:, lo:lo + w], ones_sb, pt[r],
                            start=(i == 0), stop=(i == 7), skip_group_check=True,
                        )
                    recip = sbo.tile([128, 512], F32, tag="recip", name=f"rc{h}{b}")
                    nc.vector.reciprocal(recip, ps_s)

                    ps_av = []
                    for cc in range(2):
                        ps_av.append(psav.tile([128, 512], F32, tag=f"av{cc}",
                                               name=f"psav{h}{b}{cc}"))
                    for i, r in enumerate(AVORDER):
                        lo, w = INTERVALS[r]
                        for cc in range(2):
                            nc.tensor.matmul(
                                ps_av[cc][:, lo:lo + w],
                                vT_l[4 * b + r][:, 128 * cc:128 * (cc + 1)], pt[r],
                                start=(i == 0), stop=(i == 7), skip_group_check=True,
                            )
                    relu_sb = []
                    for cc in range(2):
                        t = sbo.tile([128, 512], F32R, tag=f"relu{cc}",
                                     name=f"relu{h}{b}{cc}")
                        nc.vector.tensor_scalar_max(t, ps_av[cc], 0.0)
                        relu_sb.append(t)

                    for oc in range(4):
                        ps_o = pso.tile([128, 512], F32, tag="o", name=f"pso{h}{b}{oc}")
                        for cc in range(2):
                            nc.tensor.matmul(
                                ps_o, woT_sb[cc][:, 128 * oc:128 * (oc + 1)],
                                relu_sb[cc], start=(cc == 0), stop=(cc == 1),
                            )
                        o_sb = sbo.tile([128, 512], F32, tag="osb", name=f"o{h}{b}{oc}")
                        nc.vector.scalar_tensor_tensor(
                            o_sb, ps_o, 0.0, recip, op0=Alu.bypass, op1=Alu.mult
                        )
                        nc.sync.dma_start(
                            out=out_d.ap()[128 * oc:128 * (oc + 1),
                                           LH * h + 512 * b: LH * h + 512 * (b + 1)],
                            in_=o_sb,
                        )

                pending = []
                for b in range(BPH):
                    if b == 0:
                        if h == 0:
                            k_group(0)
                        k_group(1)
                        q_group(0)
                        vT_group(range(0, 8) if h == 0 else range(4, 8))
                    else:
                        k_group(b + 1)
                        q_group(b)
                        vT_group(range(4 * b + 4, 4 * b + 8))
                    pending.append(emit_energy(h, b))
                    if len(pending) > 1:
                        emit_tail(pending.pop(0))
                # flush before the next half's projections overwrite q/k/vT
                for ctx in pending:
                    emit_tail(ctx)
                prev_vT = vT_sb
                if h == 0:
                    # stash the k halo overlap for the next half (SBUF->SBUF
                    # DMA, off-engine); half1's block 0 reads it directly
                    kh_prev = []
                    for cc in range(2):
                        tkh = qkv.tile([128, 512], F32R, tag=f"kh{cc}",
                                       name=f"kh{cc}")
                        nc.sync.dma_start(out=tkh,
                                          in_=k_sb[cc][:, LH:LH + 512])
                        kh_prev.append(tkh)
    nc.compile()
    return nc


_NC_CACHE = {}


def _get_nc():
    if "nc" not in _NC_CACHE:
        _NC_CACHE["nc"] = _build_program()
    return _NC_CACHE["nc"]


def make_in_maps(x1, mask, Wq, bq, Wk, bk, Wv, bv, Wo, bo):
    x1 = np.asarray(x1, dtype=np.float32).reshape(CIN, L)
    wqT = _round_f32r(np.asarray(Wq, np.float32).T)
    wkT = _round_f32r(np.asarray(Wk, np.float32).T)
    wvT = _round_f32r(np.asarray(Wv, np.float32).T)
    woT = _round_f32r(np.asarray(Wo, np.float32).T)
    bq2 = np.asarray(bq, np.float32).reshape(2, 128, 1)
    bk2 = np.asarray(bk, np.float32).reshape(2, 128, 1)
    bvr = np.ascontiguousarray(
        np.broadcast_to(np.asarray(bv, np.float32).reshape(1, C), (128, C))
    )
    ones = np.ones((128, 128), np.float32)
    ident = np.eye(128, dtype=np.float32)

    try:
        import ml_dtypes
        bf16 = ml_dtypes.bfloat16
    except ImportError:  # pragma: no cover
        import jax.numpy as jnp
        bf16 = jnp.bfloat16

    if MASK_MODE == "pe":
        mint = np.concatenate(
            [_mask_tile(r, lo, w, False) for r, (lo, w) in enumerate(INTERVALS)], axis=1
        ).astype(bf16)
        m_first_int = mint[:, :512].copy()
        m_last_int = mint[:, MOFF[6]:MTOT].copy()
        m_all_pad = np.full((128, 512), NEG, np.float32).astype(bf16)

    pad_ones = np.ones((2, 128, 1), np.float32)
    pad_zeros = np.zeros((2, 128, 1), np.float32)

    in_maps = []
    for c in range(NCORES):
        g0 = LC * c - HALF
        x1h = np.zeros((CIN, HALO), np.float32)
        s0, s1 = max(g0, 0), min(g0 + HALO, L)
        x1h[:, s0 - g0:s1 - g0] = x1[:, s0:s1]
        m = {
            "x1h": _round_f32r(x1h),
            "wqT": wqT, "wkT": wkT, "wvT": wvT, "woT": woT,
            "bq2": bq2, "bk2": bk2, "bvr": bvr,
            "ones": ones,
        }
        if MASK_MODE == "pe":
            m["ident"] = ident.astype(bf16)
            m["mT_int"] = mint
            m["mT_first"] = m_all_pad if c == 0 else m_first_int
            m["mT_last"] = m_all_pad if c == NCORES - 1 else m_last_int
        else:
            m["padf"] = pad_zeros if c == 0 else pad_ones
            m["padl"] = pad_zeros if c == NCORES - 1 else pad_ones
        in_maps.append(m)
    return in_maps


def postprocess(results, mask, bo):
    cols = np.concatenate([results[c]["out"] for c in range(NCORES)], axis=1)
    out = cols[None] + np.asarray(bo, np.float32)[None, :, None]
    return (out * np.asarray(mask, np.float32)).astype(np.float32)


def kernel(x1, x2, mask, Wq, bq, Wk, bk, Wv, bv, Wo, bo, **_unused):
    from concourse.bass_utils import run_bass_kernel_spmd

    nc = _get_nc()
    in_maps = make_in_maps(x1, mask, Wq, bq, Wk, bk, Wv, bv, Wo, bo)
    res = run_bass_kernel_spmd(nc, in_maps, core_ids=list(range(NCORES)))
    return postprocess(res.results, mask, bo)


# revision 74
# speedup vs baseline: 1.0003x; 1.0003x over previous
"""Sliding-window (banded) attention for nn_AttLayer on 8 Trainium2 NeuronCores.

Reference computation (per window-block n of 512 positions, 64 blocks over L=32768):
  q/k/v = 1x1-conv projections of x1 (512ch -> 256ch)
  energy[l, m] = (q_block[:, l] . k_window[:, m]) / 16   over a 1024-wide window
  attn = softmax(energy + log(band_mask + 1e-6)) * band_mask
  out  = relu(v_window @ attn^T) -> 1x1-conv (256 -> 512) + bias, masked

Sharding: 64 blocks split contiguously across 8 cores (8 blocks each). Each core
gets a zero-padded halo slice of x1 and computes its 4096 output columns.

Kernel strategy (per core, SPMD — all per-core variation is in the data):
  - Projections computed on PE with float32r (fp32 with 12-bit significand;
    matmul is exact for pre-rounded inputs). q/k natural layout (c on
    partitions), v projected directly TRANSPOSED (positions on partitions) so
    the attention AV matmul needs no transposes.
  - energy computed transposed: energyT[m, l] = k_chunk^T q  (PE), only over
    the ~62% of 128x128 tiles that intersect the band (padded to N>=256).
  - Band masking (MASK_MODE="gpsimd"): affine_select on the otherwise-idle
    GPSIMD engine zeroes out-of-band exp values; sequence-edge padding is
    handled with per-core 0/1 data vectors so the program stays SPMD.
    (MASK_MODE="pe" alternative: additive ln(1e-6) mask preloaded into PSUM
    via a bf16 identity matmul, exactly matching the reference's +1e-6 terms.)
  - exp on ScalarE (free scale=1/16), denominators via an all-ones f32r matmul
    (column sums land replicated across partitions), reciprocal on VectorE.
  - AV + output projection on PE; normalization fused into the PSUM->SBUF
    eviction with scalar_tensor_tensor. Final bias/mask applied on host.
  - Blocks are software-pipelined (block b's colsum/AV/outproj emitted after
    block b+1's energy+exp) so PE never waits on the softmax chain, and the
    k/q/vT projection groups are interleaved INTO the block stream: the
    k-projection alone would consume x1 above the HBM wire rate, so each
    fresh-x-hungry k group is followed by work on already-resident data.
  - Halo reuse between the two halves: half 1's first four vT tiles alias
    half 0's last four (identical x1 columns), and half 0's k overlap is
    stashed via an SBUF->SBUF DMA so half 1 skips its first k-group.
"""

import numpy as np

NCORES = 8
L = 32768
CIN = 512
C = 256
BL = 512
HALF = 256
LC = L // NCORES              # 4096 positions per core
HALO = LC + 2 * HALF          # 4608
NBH = 2                       # halves per core
LH = LC // NBH                # 2048 positions per half
KSPAN = LH + 2 * HALF         # 2560 k/v positions per half
BPH = 4                       # blocks per half
SCALE = 1.0 / 16.0
NEG = float(np.log(1e-6) / SCALE)   # additive raw-energy mask ~= -221.048

# Per m-chunk r' (8 chunks of the 1024-wide window): padded valid l-interval
# (lo, width) within the block's 512 queries, all widths >= 256 for f32r speed.
INTERVALS = [
    (0, 256), (0, 256), (0, 384), (0, 512),
    (0, 512), (128, 384), (256, 256), (256, 256),
]
MOFF = np.cumsum([0] + [w for _, w in INTERVALS]).tolist()  # offsets into mT_int
MTOT = MOFF[-1]  # 2816
# accumulation order: r'=3 covers the full [0,512) so it goes first (start=True)
AVORDER = [3, 4, 2, 5, 1, 6, 0, 7]
# "pe": additive log-mask preloaded into PSUM via bf16 identity matmul.
# "gpsimd": band-mask applied post-exp with affine_select on the idle Pool
#           engine (masked terms become exact zeros); edges via per-core data.
MASK_MODE = "gpsimd"


def _round_f32r(x):
    # round-to-nearest into the f32r grid (fp32 with low 12 mantissa bits zero)
    b = np.ascontiguousarray(x, dtype=np.float32).view(np.uint32)
    return ((b + np.uint32(0x800)) & np.uint32(0xFFFFF000)).view(np.float32)


def _mask_tile(r, lo, w, all_pad):
    if all_pad:
        return np.full((128, w), NEG, dtype=np.float32)
    m = np.arange(128 * r, 128 * r + 128, dtype=np.int64)[:, None]
    l = np.arange(lo, lo + w, dtype=np.int64)[None, :]
    valid = (m - l >= 0) & (m - l <= BL - 1)
    return np.where(valid, 0.0, NEG).astype(np.float32)


def _build_program():
    import concourse.mybir as mybir
    from concourse import bacc
    from concourse.tile import TileContext

    F32 = mybir.dt.float32
    F32R = mybir.dt.float32r
    BF16 = mybir.dt.bfloat16
    Alu = mybir.AluOpType
    Act = mybir.ActivationFunctionType

    nc = bacc.Bacc()

    x1h_d = nc.dram_tensor("x1h", [CIN, HALO], F32R, kind="ExternalInput")
    wqT_d = nc.dram_tensor("wqT", [128, 4 * C], F32R, kind="ExternalInput")
    wkT_d = nc.dram_tensor("wkT", [128, 4 * C], F32R, kind="ExternalInput")
    wvT_d = nc.dram_tensor("wvT", [128, 4 * C], F32R, kind="ExternalInput")
    woT_d = nc.dram_tensor("woT", [C, CIN], F32R, kind="ExternalInput")
    bq_d = nc.dram_tensor("bq2", [2, 128, 1], F32, kind="ExternalInput")
    bk_d = nc.dram_tensor("bk2", [2, 128, 1], F32, kind="ExternalInput")
    bvr_d = nc.dram_tensor("bvr", [128, C], F32, kind="ExternalInput")
    ones_d = nc.dram_tensor("ones", [128, 128], F32R, kind="ExternalInput")
    if MASK_MODE == "pe":
        ident_d = nc.dram_tensor("ident", [128, 128], BF16, kind="ExternalInput")
        mint_d = nc.dram_tensor("mT_int", [128, MTOT], BF16, kind="ExternalInput")
        mfirst_d = nc.dram_tensor("mT_first", [128, 512], BF16, kind="ExternalInput")
        mlast_d = nc.dram_tensor("mT_last", [128, 512], BF16, kind="ExternalInput")
    else:
        padf_d = nc.dram_tensor("padf", [2, 128, 1], F32, kind="ExternalInput")
        padl_d = nc.dram_tensor("padl", [2, 128, 1], F32, kind="ExternalInput")
    out_d = nc.dram_tensor("out", [CIN, LC], F32, kind="ExternalOutput")

    with TileContext(nc) as tc:
        with (
            tc.tile_pool(name="consts", bufs=1) as consts,
            tc.tile_pool(name="xpool", bufs=1) as xpool,
            tc.tile_pool(name="qkv", bufs=1) as qkv,
            tc.tile_pool(name="ptp", bufs=2) as ptp,
            tc.tile_pool(name="sbo", bufs=4) as sbo,
            tc.tile_pool(name="pse", bufs=3, space="PSUM") as pse,
            tc.tile_pool(name="pss", bufs=1, space="PSUM") as pss,
            tc.tile_pool(name="psav", bufs=1, space="PSUM") as psav,
            tc.tile_pool(name="pso", bufs=2, space="PSUM") as pso,
        ):
            # warm the ACT exp table while DMAs stream in
            warm_sb = consts.tile([1, 8], F32)
            nc.vector.memset(warm_sb, 0.0)
            nc.scalar.activation(warm_sb, warm_sb, Act.Exp)

            # warm the PE clock gate (HAM) during the initial DMA wait:
            # dummy bf16 matmuls on memset data keep the array busy so the
            # first real projections run at the full 2.4 GHz
            warm_a = consts.tile([128, 128], BF16, name="warm_a")
            nc.vector.memset(warm_a, 1.0)
            warm_b = consts.tile([128, 512], BF16, name="warm_b")
            nc.vector.memset(warm_b, 1.0)
            for wi in range(5):
                warm_ps = pse.tile([128, 512], F32, tag="e", name=f"wps{wi}")
                nc.tensor.matmul(warm_ps, warm_a, warm_b, start=True, stop=True)

            # critical-path-first DMA order: the first PE work is the h=0
            # k-projection of columns [0:512), needing wkT/bk and x chunk 0;
            # pair (wkT[kc], x[kc]) so the accumulation group streams in
            wT_sb = {}
            wk_all = consts.tile([128, 4 * C], F32R, name="wk_all")
            nc.sync.dma_start(out=wk_all, in_=wkT_d.ap())
            x_sb_h0 = []
            for kc in range(4):
                wT_sb[("k", kc)] = wk_all[:, C * kc:C * (kc + 1)]
                tx = xpool.tile([128, KSPAN], F32R, tag=f"x{kc}", name=f"x{kc}_0")
                x_sb_h0.append(tx)
                nc.sync.dma_start(
                    out=tx[:, 0:512],
                    in_=x1h_d.ap()[128 * kc:128 * (kc + 1), 0:512],
                )
            bk_sb = []
            for cc in range(2):
                tk = consts.tile([128, 1], F32, name=f"bk{cc}")
                nc.sync.dma_start(out=tk, in_=bk_d.ap()[cc])
                bk_sb.append(tk)
            def _x0_pair(ct):
                for kc in range(4):
                    nc.sync.dma_start(
                        out=x_sb_h0[kc][:, 512 * ct:512 * (ct + 2)],
                        in_=x1h_d.ap()[128 * kc:128 * (kc + 1),
                                       512 * ct:512 * (ct + 2)],
                    )

            _x0_pair(1)
            wq_all = consts.tile([128, 4 * C], F32R, name="wq_all")
            nc.sync.dma_start(out=wq_all, in_=wqT_d.ap())
            for kc in range(4):
                wT_sb[("q", kc)] = wq_all[:, C * kc:C * (kc + 1)]
            bq_sb = []
            for cc in range(2):
                tq = consts.tile([128, 1], F32, name=f"bq{cc}")
                nc.sync.dma_start(out=tq, in_=bq_d.ap()[cc])
                bq_sb.append(tq)
            wv_all = consts.tile([128, 4 * C], F32R, name="wv_all")
            nc.sync.dma_start(out=wv_all, in_=wvT_d.ap())
            for kc in range(4):
                wT_sb[("v", kc)] = wv_all[:, C * kc:C * (kc + 1)]
            bvrep_sb = consts.tile([128, C], F32)
            nc.sync.dma_start(out=bvrep_sb, in_=bvr_d.ap())
            _x0_pair(3)

            ones_sb = consts.tile([128, 128], F32R)
            nc.sync.dma_start(out=ones_sb, in_=ones_d.ap())
            if MASK_MODE == "pe":
                ident_sb = consts.tile([128, 128], BF16)
                nc.sync.dma_start(out=ident_sb, in_=ident_d.ap())
                mint_sb = consts.tile([128, MTOT], BF16)
                nc.sync.dma_start(out=mint_sb, in_=mint_d.ap())
                mfirst_sb = consts.tile([128, 512], BF16)
                nc.sync.dma_start(out=mfirst_sb, in_=mfirst_d.ap())
                mlast_sb = consts.tile([128, 512], BF16)
                nc.sync.dma_start(out=mlast_sb, in_=mlast_d.ap())
            else:
                padf_sb, padl_sb = [], []
                for r in range(2):
                    tf = consts.tile([128, 1], F32, name=f"padf{r}")
                    nc.sync.dma_start(out=tf, in_=padf_d.ap()[r])
                    padf_sb.append(tf)
                    tl = consts.tile([128, 1], F32, name=f"padl{r}")
                    nc.sync.dma_start(out=tl, in_=padl_d.ap()[r])
                    padl_sb.append(tl)
            woT_sb = []
            for cc in range(2):
                t = consts.tile([128, CIN], F32R, name=f"woT{cc}")
                nc.sync.dma_start(out=t, in_=woT_d.ap()[128 * cc:128 * (cc + 1), :])
                woT_sb.append(t)

            for h in range(NBH):
                base = LH * h  # halo-coord start of this half's x1/k/v span
                if h == 0:
                    x_sb = x_sb_h0
                else:
                    x_sb = []
                    for kc in range(4):
                        t = xpool.tile([128, KSPAN], F32R, tag=f"x{kc}",
                                       name=f"x{kc}_{h}")
                        x_sb.append(t)
                # split per 512-column chunk so projections start while the
                # rest of the slice streams in (all h=0 chunks issued up top)
                if h > 0:
                    for kc in range(4):
                        nc.sync.dma_start(
                            out=x_sb[kc][:, 256:1536],
                            in_=x1h_d.ap()[128 * kc:128 * (kc + 1),
                                           base + 256:base + 1536],
                        )
                    for kc in range(4):
                        nc.sync.dma_start(
                            out=x_sb[kc][:, 1536:2560],
                            in_=x1h_d.ap()[128 * kc:128 * (kc + 1),
                                           base + 1536:base + 2560],
                        )

                # ---- projections ----
                q_sb, k_sb = [], []
                for cc in range(2):
                    q_sb.append(qkv.tile([128, LH], F32R, tag=f"q{cc}", name=f"q{cc}_{h}"))
                    k_sb.append(qkv.tile([128, KSPAN], F32R, tag=f"k{cc}", name=f"k{cc}_{h}"))
                # projection group emitters; actual emission is interleaved
                # with the attention blocks below so the k-projection's burst
                # demand for fresh x chunks never outruns the DMA wire rate
                def k_group(mt):
                    for cc in range(2):
                        csl = slice(128 * cc, 128 * (cc + 1))
                        ps = pse.tile([128, 512], F32, tag="e",
                                      name=f"psk{h}{cc}{mt}")
                        for kc in range(4):
                            nc.tensor.matmul(
                                ps, wT_sb[("k", kc)][:, csl],
                                x_sb[kc][:, 512 * mt:512 * (mt + 1)],
                                start=(kc == 0), stop=(kc == 3),
                            )
                        nc.vector.tensor_scalar_add(
                            k_sb[cc][:, 512 * mt:512 * (mt + 1)], ps, bk_sb[cc]
                        )

                def q_group(lt):
                    for cc in range(2):
                        csl = slice(128 * cc, 128 * (cc + 1))
                        ps = pse.tile([128, 512], F32, tag="e",
                                      name=f"psq{h}{cc}{lt}")
                        for kc in range(4):
                            nc.tensor.matmul(
                                ps, wT_sb[("q", kc)][:, csl],
                                x_sb[kc][:, HALF + 512 * lt: HALF + 512 * (lt + 1)],
                                start=(kc == 0), stop=(kc == 3),
                            )
                        nc.vector.tensor_scalar_add(
                            q_sb[cc][:, 512 * lt:512 * (lt + 1)], ps, bq_sb[cc]
                        )

                vT_sb = [None] * (KSPAN // 128)
                if h > 0:
                    # halo reuse: this half's m=0..3 v-chunks cover the same
                    # x1 columns as the previous half's m=16..19 — alias them
                    for mt in range(4):
                        vT_sb[mt] = prev_vT[16 + mt]

                def vT_group(mts):
                    for mt in mts:
                        ps = pso.tile([128, C], F32, tag="o", name=f"psv{h}{mt}")
                        for kc in range(4):
                            nc.tensor.matmul(
                                ps, x_sb[kc][:, 128 * mt:128 * (mt + 1)],
                                wT_sb[("v", kc)], start=(kc == 0), stop=(kc == 3),
                            )
                        t = qkv.tile([128, C], F32R, tag=f"v{mt}", name=f"vT{mt}_{h}")
                        # eviction with the (per-free-element) v bias folded in
                        nc.vector.tensor_tensor(t, ps, bvrep_sb, op=Alu.add)
                        vT_sb[mt] = t

                # ---- attention blocks (software-pipelined: block b's
                # colsum/AV/outproj are emitted after block b+1's energy+exp
                # so PE never waits on the ACT/Pool softmax chain) ----
                def emit_energy(h, b, k_sb=k_sb, q_sb=q_sb, vT_sb=vT_sb,
                                kh=(kh_prev if h > 0 else None)):
                    woff = 512 * b   # window start in k/vT coords
                    first_blk = (h == 0 and b == 0)
                    last_blk = (h == NBH - 1 and b == BPH - 1)
                    pt = {}
                    for r in AVORDER:
                        lo, w = INTERVALS[r]
                        ps_e = pse.tile([128, w], F32, tag="e", name=f"pse{h}{b}{r}")
                        if MASK_MODE == "pe":
                            if first_blk and r < 2:
                                msrc = mfirst_sb[:, 256 * r:256 * r + w]
                            elif last_blk and r >= 6:
                                msrc = mlast_sb[:, 256 * (r - 6):256 * (r - 6) + w]
                            else:
                                msrc = mint_sb[:, MOFF[r]:MOFF[r] + w]
                            nc.tensor.matmul(ps_e, ident_sb, msrc, start=True,
                                             stop=False, skip_group_check=True)
                        for cc in range(2):
                            if kh is not None and b == 0 and r < 4:
                                klhs = kh[cc][:, 128 * r:128 * (r + 1)]
                            else:
                                klhs = k_sb[cc][:, woff + 128 * r:
                                                woff + 128 * (r + 1)]
                            nc.tensor.matmul(
                                ps_e, klhs,
                                q_sb[cc][:, 512 * b + lo: 512 * b + lo + w],
                                start=(MASK_MODE != "pe" and cc == 0),
                                stop=(cc == 1), skip_group_check=True,
                            )
                        t = ptp.tile([128, w], F32R, tag=f"pt{r}", name=f"pt{r}_{h}{b}")
                        nc.scalar.activation(t, ps_e, Act.Exp, scale=SCALE)
                        if MASK_MODE == "gpsimd":
                            # zero outside the band: one affine compare per tile
                            # (lower bound bites for r<=3, upper for r>=4)
                            if r <= 3:
                                nc.gpsimd.affine_select(
                                    out=t, in_=t, compare_op=Alu.is_ge, fill=0.0,
                                    base=128 * r - lo, channel_multiplier=1,
                                    pattern=[[-1, w]],
                                )
                            else:
                                # valid iff (128r+m')-l <= 511, recast as
                                # (511-128r+lo) - m' + j >= 0 (is_ge only)
                                nc.gpsimd.affine_select(
                                    out=t, in_=t, compare_op=Alu.is_ge, fill=0.0,
                                    base=(BL - 1) - 128 * r + lo,
                                    channel_multiplier=-1,
                                    pattern=[[1, w]],
                                )
                            if first_blk and r < 2:
                                nc.vector.tensor_scalar_mul(t, t, padf_sb[r])
                            elif last_blk and r >= 6:
                                nc.vector.tensor_scalar_mul(t, t, padl_sb[r - 6])
                        pt[r] = t
                    return (h, b, pt, vT_sb)

                def emit_tail(ctx):
                    h, b, pt, vT_l = ctx
                    ps_s = pss.tile([128, 512], F32, tag="s", name=f"pss{h}{b}")
                    for i, r in enumerate(AVORDER):
                        lo, w = INTERVALS[r]
                        nc.tensor.matmul(
                            ps_s[:, lo:lo + w], ones_sb, pt[r],
                            start=(i == 0), stop=(i == 7), skip_group_check=True,
                        )
                    recip = sbo.tile([128, 512], F32, tag="recip", name=f"rc{h}{b}")
                    nc.vector.reciprocal(recip, ps_s)

                    ps_av = []
                    for cc in range(2):
                        ps_av.append(psav.tile([128, 512], F32, tag=f"av{cc}",
                                               name=f"psav{h}{b}{cc}"))
                    for i, r in enumerate(AVORDER):
                        lo, w = INTERVALS[r]
                        for cc in range(2):
                            nc.tensor.matmul(
                                ps_av[cc][:, lo:lo + w],
                                vT_l[4 * b + r][:, 128 * cc:128 * (cc + 1)], pt[r],
                                start=(i == 0), stop=(i == 7), skip_group_check=True,
                            )
                    relu_sb = []
                    for cc in range(2):
                        t = sbo.tile([128, 512], F32R, tag=f"relu{cc}",
                                     name=f"relu{h}{b}{cc}")
                        nc.vector.tensor_scalar_max(t, ps_av[cc], 0.0)
                        relu_sb.append(t)

                    for oc in range(4):
                        ps_o = pso.tile([128, 512], F32, tag="o", name=f"pso{h}{b}{oc}")
                        for cc in range(2):
                            nc.tensor.matmul(
                                ps_o, woT_sb[cc][:, 128 * oc:128 * (oc + 1)],
                                relu_sb[cc], start=(cc == 0), stop=(cc == 1),
                            )
                        o_sb = sbo.tile([128, 512], F32, tag="osb", name=f"o{h}{b}{oc}")
                        nc.vector.scalar_tensor_tensor(
                            o_sb, ps_o, 0.0, recip, op0=Alu.bypass, op1=Alu.mult
                        )
                        nc.sync.dma_start(
                            out=out_d.ap()[128 * oc:128 * (oc + 1),
                                           LH * h + 512 * b: LH * h + 512 * (b + 1)],
                            in_=o_sb,
                        )

                pending = []
                for b in range(BPH):
                    if b == 0:
                        if h == 0:
                            k_group(0)
                        k_group(1)
                        q_group(0)
                        vT_group(range(0, 8) if h == 0 else range(4, 8))
                    else:
                        k_group(b + 1)
                        q_group(b)
                        vT_group(range(4 * b + 4, 4 * b + 8))
                    pending.append(emit_energy(h, b))
                    if len(pending) > 1:
                        emit_tail(pending.pop(0))
                # flush before the next half's projections overwrite q/k/vT
                for ctx in pending:
                    emit_tail(ctx)
                prev_vT = vT_sb
                if h == 0:
                    # stash the k halo overlap for the next half (SBUF->SBUF
                    # DMA, off-engine); half1's block 0 reads it directly
                    kh_prev = []
                    for cc in range(2):
                        tkh = qkv.tile([128, 512], F32R, tag=f"kh{cc}",
                                       name=f"kh{cc}")
                        nc.sync.dma_start(out=tkh,
                                          in_=k_sb[cc][:, LH:LH + 512])
                        kh_prev.append(tkh)
    nc.compile()
    return nc


_NC_CACHE = {}


def _get_nc():
    if "nc" not in _NC_CACHE:
        _NC_CACHE["nc"] = _build_program()
    return _NC_CACHE["nc"]


def make_in_maps(x1, mask, Wq, bq, Wk, bk, Wv, bv, Wo, bo):
    x1 = np.asarray(x1, dtype=np.float32).reshape(CIN, L)
    def _ileave(w):
        # (512, 256) -> (128, 4*256): chunk kc at columns [256*kc, 256*(kc+1))
        wt = _round_f32r(np.asarray(w, np.float32).T)
        return np.ascontiguousarray(
            wt.reshape(4, 128, C).transpose(1, 0, 2).reshape(128, 4 * C))
    wqT = _ileave(Wq)
    wkT = _ileave(Wk)
    wvT = _ileave(Wv)
    woT = _round_f32r(np.asarray(Wo, np.float32).T)
    bq2 = np.asarray(bq, np.float32).reshape(2, 128, 1)
    bk2 = np.asarray(bk, np.float32).reshape(2, 128, 1)
    bvr = np.ascontiguousarray(
        np.broadcast_to(np.asarray(bv, np.float32).reshape(1, C), (128, C))
    )
    ones = np.ones((128, 128), np.float32)
    ident = np.eye(128, dtype=np.float32)

    try:
        import ml_dtypes
        bf16 = ml_dtypes.bfloat16
    except ImportError:  # pragma: no cover
        import jax.numpy as jnp
        bf16 = jnp.bfloat16

    if MASK_MODE == "pe":
        mint = np.concatenate(
            [_mask_tile(r, lo, w, False) for r, (lo, w) in enumerate(INTERVALS)], axis=1
        ).astype(bf16)
        m_first_int = mint[:, :512].copy()
        m_last_int = mint[:, MOFF[6]:MTOT].copy()
        m_all_pad = np.full((128, 512), NEG, np.float32).astype(bf16)

    pad_ones = np.ones((2, 128, 1), np.float32)
    pad_zeros = np.zeros((2, 128, 1), np.float32)

    in_maps = []
    for c in range(NCORES):
        g0 = LC * c - HALF
        x1h = np.zeros((CIN, HALO), np.float32)
        s0, s1 = max(g0, 0), min(g0 + HALO, L)
        x1h[:, s0 - g0:s1 - g0] = x1[:, s0:s1]
        m = {
            "x1h": _round_f32r(x1h),
            "wqT": wqT, "wkT": wkT, "wvT": wvT, "woT": woT,
            "bq2": bq2, "bk2": bk2, "bvr": bvr,
            "ones": ones,
        }
        if MASK_MODE == "pe":
            m["ident"] = ident.astype(bf16)
            m["mT_int"] = mint
            m["mT_first"] = m_all_pad if c == 0 else m_first_int
            m["mT_last"] = m_all_pad if c == NCORES - 1 else m_last_int
        else:
            m["padf"] = pad_zeros if c == 0 else pad_ones
            m["padl"] = pad_zeros if c == NCORES - 1 else pad_ones
        in_maps.append(m)
    return in_maps


def postprocess(results, mask, bo):
    cols = np.concatenate([results[c]["out"] for c in range(NCORES)], axis=1)
    out = cols[None] + np.asarray(bo, np.float32)[None, :, None]
    return (out * np.asarray(mask, np.float32)).astype(np.float32)


def kernel(x1, x2, mask, Wq, bq, Wk, bk, Wv, bv, Wo, bo, **_unused):
    from concourse.bass_utils import run_bass_kernel_spmd

    nc = _get_nc()
    in_maps = make_in_maps(x1, mask, Wq, bq, Wk, bk, Wv, bv, Wo, bo)
    res = run_bass_kernel_spmd(nc, in_maps, core_ids=list(range(NCORES)))
    return postprocess(res.results, mask, bo)
